# revision 20
# baseline (speedup 1.0000x reference)
"""Trainium2 Bass kernel for nn_LitToClauseLayer (gather + segment_sum + LSTM cell).

Reference computation:
    msg   = segment_sum(x_l[edge_lit], edge_clause, num_segments=N_CLAUSE)   # [NC, D]
    gates = msg @ W_ih.T + b_ih + h0 @ W_hh.T + b_hh                         # [NC, 4D]
    i, f, g, o = split(gates); i,f,o = sigmoid; g = tanh
    c_new = f*c0 + i*g ; h_new = o*tanh(c_new)
    returns (h_new, c_new)

Distribution (8 cores, SPMD): clauses + the clause-sorted edge list sharded
across cores; x_l (fp16) and the LSTM weights replicated. Everything on
device is feature-major ([D=128 partitions, clause]).

Pipeline per 14-chunk superchunk (28 per core):
  - Edge rows are fetched with the SWDGE dma_gather ucode. x_l is split in
    4 row-banks (int16 index range); per (superchunk, bank) the edges are
    packed into calls of <= 1024 indices (the ucode's hard per-call limit)
    and the calls are emitted round-robin across the 4 SWDGE queues so a
    queue's ring drain overlaps the other queues' descriptor generation.
    Per-(chunk, bank) slot budgets are the max over cores (SPMD), rounded
    to 128-slot tiles; pads use scattered dummy rows.
  - The segment sum is msgT += xg_tile^T @ onehot_tile per 128-edge tile.
    The one-hot ([p, t, c] = 1 iff eloc[p,t]==c) is built in one DVE
    is_equal per superchunk using a duplicated eloc table ([p,2t]==[p,2t+1])
    so both operands have an inner stride-1 pair dim -> DVE 2x mode, while
    the output stays contiguous for the matmul rhs.
  - Gate matmuls are 384/512-wide per gate per chunk-group; gate PSUM is
    two [P, 2, 512] half-tiles (1 bank per gate block, pool bufs=3) so the
    next group's matmuls overlap the previous group's activations.
  - Activations run per gate per group with per-partition bias APs;
    tanh(c_new) and the pointwise LSTM update run superchunk-wide on
    ScalarE/DVE; h/c outputs stream out per superchunk.
"""

import numpy as np

N_LIT, N_CLAUSE, N_EDGES, D = 100000, 400000, 1200000, 128
N_CORES = 8
CPC = N_CLAUSE // N_CORES  # clauses per core (50000)
P = 128
BANK_ROWS = 25000
SC_CHUNKS = 14             # chunks per superchunk (392 = 14*28)
GROUPS = [4, 4, 3, 3]      # chunks per gate-matmul group (PSUM gate = 1 bank)
MAX_CALL_IDX = 1024        # per-call idx cap (ucode hard limit)
SCRATCH = 65536            # bigger descriptor carveout

_cache = {}

# test-harness hooks (ignored in normal use)
TRACE = False
LAST_RESULT = None


def _compute_structure(edge_lit, edge_clause, n_lit=N_LIT, cpc=CPC,
                       bank_rows=BANK_ROWS, sc_chunks=SC_CHUNKS,
                       n_cores=N_CORES):
    """Shared (SPMD) program structure + per-core edge placement."""
    n_banks = -(-n_lit // bank_rows)
    n_chunks = -(-cpc // P)
    n_chunks = -(-n_chunks // sc_chunks) * sc_chunks
    n_sc = n_chunks // sc_chunks

    counts = np.zeros((n_cores, n_chunks, n_banks), dtype=np.int64)
    per_core = []
    for k in range(n_cores):
        marks = k * cpc + P * np.arange(n_chunks + 1)
        marks = np.minimum(marks, (k + 1) * cpc)
        bounds = np.searchsorted(edge_clause, marks, side="left")
        cells = {}
        for j in range(n_chunks):
            e0, e1 = bounds[j], bounds[j + 1]
            if e1 > e0:
                lits = edge_lit[e0:e1]
                locs = (edge_clause[e0:e1] - (k * cpc + j * P)).astype(np.int16)
                banks = lits // bank_rows
                order = np.argsort(banks, kind="stable")
                lits, locs, banks = lits[order], locs[order], banks[order]
                for b in range(n_banks):
                    m = banks == b
                    if m.any():
                        cells[(j, b)] = (lits[m] - b * bank_rows, locs[m])
                        counts[k, j, b] = int(m.sum())
        per_core.append(cells)

    maxc = counts.max(axis=0)                       # [n_chunks, n_banks]
    budgets = ((maxc + P - 1) // P) * P             # slots per cell
    for j in range(n_chunks):                       # every chunk >= 1 tile
        if budgets[j].sum() == 0:
            budgets[j, 0] = P

    structure = {
        "n_lit": n_lit, "n_banks": n_banks, "bank_rows": bank_rows,
        "n_chunks": n_chunks, "n_sc": n_sc, "sc_chunks": sc_chunks,
        "budgets": budgets,
    }
    return structure, per_core


def _sc_list(st):
    """Variable superchunk sizes: full-size scs, then a shrinking tail so
    the post-final-gather drain covers few chunks. Entries are
    (start_chunk, scc_s, groups_s)."""
    n_chunks, scc = st["n_chunks"], st["sc_chunks"]
    full = n_chunks // scc - 1          # leave one sc's worth for the tail
    lst = [(k * scc, scc, [4, 4, 3, 3]) for k in range(full)]
    start = full * scc
    rem = n_chunks - start              # == scc == 14
    assert rem == 14
    lst.append((start, 10, [4, 3, 3]))
    lst.append((start + 10, 4, [4]))
    return lst


def _geometry(st):
    """Derive per-sc call/tile geometry from budgets.

    Per (sc, bank): cells packed greedily into gather calls <= MAX_CALL_IDX.
    Tile order within an sc: bank-major, then chunk (cell) order; ti indexes
    the sc's one-hot columns and eloc slots.
    """
    n_banks = st["n_banks"]
    budgets = st["budgets"]
    sc_list = _sc_list(st)
    sc_geom = []
    icol = 0        # global idx-table column (int16 cols, 16 idx per col)
    tcol = 0        # global tile column (eloc)
    max_nt = 0
    max_bank_nt = [0] * n_banks
    for start, scc_s, _groups in sc_list:
        calls = []   # (bank, icol_base, nidx, t0_in_bank, ntile)
        tiles = []   # (bank, t_in_bank, chunk_local) in ti order
        for b in range(n_banks):
            t_in_bank = 0
            cur_nidx = 0
            cur_icol = icol
            cur_t0 = 0
            for c in range(scc_s):
                j = start + c
                bud = int(budgets[j, b])
                if bud == 0:
                    continue
                if cur_nidx + bud > MAX_CALL_IDX and cur_nidx > 0:
                    calls.append((b, cur_icol, cur_nidx, cur_t0,
                                  t_in_bank - cur_t0))
                    icol += cur_nidx // 16
                    cur_icol = icol
                    cur_nidx = 0
                    cur_t0 = t_in_bank
                cur_nidx += bud
                for _ in range(bud // P):
                    tiles.append((b, t_in_bank, c))
                    t_in_bank += 1
            if cur_nidx > 0:
                calls.append((b, cur_icol, cur_nidx, cur_t0,
                              t_in_bank - cur_t0))
                icol += cur_nidx // 16
            max_bank_nt[b] = max(max_bank_nt[b], t_in_bank)
        sc_geom.append({"calls": calls, "tiles": tiles, "tbase": tcol})
        tcol += len(tiles)
        max_nt = max(max_nt, len(tiles))
    return {
        "sc_geom": sc_geom,
        "total_icols": icol,
        "total_tiles": tcol,
        "max_nt": max_nt,
        "max_bank_nt": max_bank_nt,
    }


def _build_program(st, geom):
    import concourse.bacc as bacc
    import concourse.bass as bass
    import concourse.mybir as mybir
    import concourse.tile as tile

    dt = mybir.dt
    n_chunks, n_sc, scc = st["n_chunks"], st["n_sc"], st["sc_chunks"]
    n_banks, bank_rows = st["n_banks"], st["bank_rows"]
    n_lit = st["n_lit"]
    budgets = st["budgets"]
    ncols = n_chunks * P
    sc_cols = scc * P
    sc_geom = geom["sc_geom"]
    max_nt = geom["max_nt"]
    max_bank_nt = geom["max_bank_nt"]
    total_icols = geom["total_icols"]
    total_tiles = geom["total_tiles"]

    sc_list = _sc_list(st)
    assert sum(s[1] for s in sc_list) == n_chunks

    nc = bacc.Bacc(None, target_bir_lowering=False, num_swdge_queues=4,
                   dynamic_dma_scratch_size=SCRATCH)

    f32, fp16, i16 = dt.float32, dt.float16, dt.int16
    Sig = mybir.ActivationFunctionType.Sigmoid
    Tnh = mybir.ActivationFunctionType.Tanh

    xlb = nc.dram_tensor("xlb", [n_lit, D], fp16, kind="ExternalInput")
    h0t = nc.dram_tensor("h0t", [P, ncols], fp16, kind="ExternalInput")
    c0t = nc.dram_tensor("c0t", [P, ncols], fp16, kind="ExternalInput")
    eidx = nc.dram_tensor("eidx", [P, total_icols], i16, kind="ExternalInput")
    eloc = nc.dram_tensor("eloc", [P, 2 * total_tiles], fp16, kind="ExternalInput")
    wih = nc.dram_tensor("wih", [P, 4 * D], fp16, kind="ExternalInput")
    whh = nc.dram_tensor("whh", [P, 4 * D], fp16, kind="ExternalInput")
    btab = nc.dram_tensor("btab", [P, 4], f32, kind="ExternalInput")
    iotaf = nc.dram_tensor("iotaf", [P, P], fp16, kind="ExternalInput")
    ht = nc.dram_tensor("ht", [P, ncols], dt.float16, kind="ExternalOutput")
    ct = nc.dram_tensor("ct", [P, ncols], dt.float16, kind="ExternalOutput")

    with tile.TileContext(nc) as tc:
        with (
            tc.tile_pool(name="const", bufs=1) as cpool,
            tc.tile_pool(name="tabs", bufs=3) as tabs,
            tc.tile_pool(name="gat", bufs=2) as gat,
            tc.tile_pool(name="oh", bufs=2) as ohp,
            tc.tile_pool(name="hc", bufs=2) as hcp,
            tc.tile_pool(name="msg", bufs=3) as msgp,
            tc.tile_pool(name="acts", bufs=2) as actp,
            tc.tile_pool(name="outs", bufs=2) as outp,
            tc.tile_pool(name="pm", bufs=2, space="PSUM") as pmsum,
            tc.tile_pool(name="pg", bufs=3, space="PSUM") as pgate,
        ):
            wih_sb = cpool.tile([P, 4 * D], fp16, tag="wih")
            whh_sb = cpool.tile([P, 4 * D], fp16, tag="whh")
            b_sb = cpool.tile([P, 4], f32, tag="btab")
            iota_sb = cpool.tile([P, P], fp16, tag="iota")

            for s, (start, scc_s, groups_s) in enumerate(sc_list):
                sc_cols = scc_s * P
                col0 = start * P
                grp_c0 = np.cumsum([0] + groups_s)
                last_sc = (s == len(sc_list) - 1)
                g = sc_geom[s]
                calls, tiles, tbase = g["calls"], g["tiles"], g["tbase"]
                n_t = len(tiles)
                sc_icol0 = calls[0][1]
                sc_icols = sum(cnidx // 16 for _, _, cnidx, _, _ in calls)

                idx_t = tabs.tile([P, max(sc_icols, 16)], i16, tag="idx")
                nc.sync.dma_start(out=idx_t[:, :sc_icols],
                                  in_=eidx[:, sc_icol0:sc_icol0 + sc_icols])
                elo_t = tabs.tile([P, 2 * max_nt], fp16, tag="elo")
                nc.sync.dma_start(out=elo_t[:, :2 * n_t],
                                  in_=eloc[:, 2 * tbase:2 * (tbase + n_t)])
                if s == 0:
                    # constants go on the sync DMA queue AFTER the first
                    # superchunk's tables so the first gather isn't stuck
                    # behind ~290KB of weights; they are only consumed by
                    # the gate matmuls, well after this point.
                    nc.sync.dma_start(out=wih_sb[:], in_=wih[:, :])
                    nc.sync.dma_start(out=whh_sb[:], in_=whh[:, :])
                    nc.sync.dma_start(out=b_sb[:], in_=btab[:, :])
                    nc.sync.dma_start(out=iota_sb[:], in_=iotaf[:, :])

                g_tiles = [None] * n_banks
                # emit calls round-robin across banks so consecutive calls
                # land on different SWDGE queues (a queue's next call blocks
                # on its ring drain; interleaving overlaps gen with drain)
                by_bank = {}
                for call in calls:
                    by_bank.setdefault(call[0], []).append(call)
                rr = []
                k = 0
                while any(by_bank.values()):
                    for b in sorted(by_bank):
                        if by_bank[b]:
                            rr.append(by_bank[b].pop(0))
                calls = rr
                for b, icol0, cnidx, t0, ntile in calls:
                    if g_tiles[b] is None:
                        g_tiles[b] = gat.tile([P, max_bank_nt[b], D], fp16,
                                              tag=f"g{b}", name=f"g{b}")
                    lo = b * bank_rows
                    hi = min(lo + bank_rows, n_lit)
                    nc.gpsimd.dma_gather(
                        out_ap=g_tiles[b][:, t0:t0 + ntile, :],
                        in_ap=xlb[lo:hi, :],
                        idxs_ap=idx_t[:, icol0 - sc_icol0:
                                      icol0 - sc_icol0 + cnidx // 16],
                        num_idxs=cnidx, num_idxs_reg=cnidx, elem_size=D,
                        queue_num=b % 4)

                # one-hot oh[p, t, c] = (eloc[p, t] == c), contiguous over c
                # so the matmul rhs streams dense columns. elo_t holds each
                # value twice ([p, 2t], [p, 2t+1]); three build variants to
                # A/B on hardware (same result, different DVE access modes).
                oh_t = ohp.tile([P, max_nt, P], fp16, tag="onehot")
                i_ap = iota_sb[:]
                e_ap = elo_t[:]
                # pair trick: inner dim of 2 with stride 1 on both operands
                # (eloc stored duplicated) -> DVE 2x mode with the output in
                # the contiguous [p, t, c] layout the matmul rhs wants.
                out3 = bass.AP(oh_t[:].tensor, oh_t[:].offset,
                               [oh_t[:].ap[0], [P, n_t], [2, 64], [1, 2]])
                iota_b = bass.AP(i_ap.tensor, i_ap.offset,
                                 [i_ap.ap[0], [0, n_t], [2, 64], [1, 2]])
                elo_b = bass.AP(e_ap.tensor, e_ap.offset,
                                [e_ap.ap[0], [2, n_t], [0, 64], [1, 2]])
                nc.vector.tensor_tensor(out=out3, in0=iota_b, in1=elo_b,
                                        op=mybir.AluOpType.is_equal)

                h0_t = hcp.tile([P, sc_cols], fp16, tag="h0")
                c0_t = hcp.tile([P, sc_cols], fp16, tag="c0")
                nc.sync.dma_start(out=h0_t[:], in_=h0t[:, col0:col0 + sc_cols])
                nc.sync.dma_start(out=c0_t[:], in_=c0t[:, col0:col0 + sc_cols])

                # tiles grouped by chunk
                by_chunk = [[] for _ in range(scc_s)]
                for ti, (b, t, c) in enumerate(tiles):
                    by_chunk[c].append((b, t, ti))

                i_s = actp.tile([P, sc_cols], fp16, tag="i_s")
                f_s = actp.tile([P, sc_cols], fp16, tag="f_s")
                g_s = actp.tile([P, sc_cols], fp16, tag="g_s")
                o_s = actp.tile([P, sc_cols], fp16, tag="o_s")

                for gi, gsz in enumerate(groups_s):
                    c0g = int(grp_c0[gi])        # first chunk (local)
                    W = gsz * P
                    go = c0g * P                 # column offset within sc
                    # segment-sum for the group's chunks -> one PSUM bank
                    msg_ps = pmsum.tile([P, 4 * P], f32, tag="msgps")
                    for c in range(c0g, c0g + gsz):
                        lst = by_chunk[c]
                        cl = c - c0g
                        for k, (b, t, ti) in enumerate(lst):
                            nc.tensor.matmul(
                                out=msg_ps[:, cl * P:(cl + 1) * P],
                                lhsT=g_tiles[b][:, t, :],
                                rhs=oh_t[:, ti, :],
                                start=(k == 0),
                                stop=(k == len(lst) - 1),
                            )
                    msg_sb = msgp.tile([P, 4 * P], fp16, tag="msgsb")
                    nc.vector.tensor_copy(out=msg_sb[:, :W], in_=msg_ps[:, :W])

                    # wide gate matmuls, two half-tiles of [P, 2, 512]
                    # (each gate block = one PSUM bank; pool bufs=3 lets the
                    # next group's matmuls start while ACT drains this one)
                    h0_g = h0_t[:, go:go + W]
                    outs_sb = (i_s, f_s, g_s, o_s)
                    funcs = (Sig, Sig, Tnh, Sig)
                    for half in range(2):
                        gp = pgate.tile([P, 2, 4 * P], f32, tag="gps",
                                        name=f"gps{half}")
                        for k in range(2):
                            gate = 2 * half + k
                            nc.tensor.matmul(out=gp[:, k, :W],
                                             lhsT=wih_sb[:, gate * D:(gate + 1) * D],
                                             rhs=msg_sb[:, :W], start=True, stop=False)
                            nc.tensor.matmul(out=gp[:, k, :W],
                                             lhsT=whh_sb[:, gate * D:(gate + 1) * D],
                                             rhs=h0_g, start=False, stop=True)
                        for k in range(2):
                            gate = 2 * half + k
                            nc.scalar.activation(outs_sb[gate][:, go:go + W],
                                                 gp[:, k, :W], funcs[gate],
                                                 bias=b_sb[:, gate:gate + 1])

                # pointwise LSTM update for the whole superchunk (the tail
                # superchunks are small, so the post-final-gather drain is
                # short without further splitting)
                ct_acc = outp.tile([P, sc_cols], fp16, tag="ct_a")
                ht_acc = outp.tile([P, sc_cols], fp16, tag="ht_a")
                tnh_c = outp.tile([P, sc_cols], fp16, tag="tc_a")
                d0 = col0
                nc.vector.tensor_mul(out=f_s[:], in0=f_s[:], in1=c0_t[:])
                nc.vector.tensor_mul(out=i_s[:], in0=i_s[:], in1=g_s[:])
                nc.vector.tensor_add(out=ct_acc[:], in0=f_s[:], in1=i_s[:])
                nc.scalar.activation(tnh_c[:], ct_acc[:], Tnh)
                nc.vector.tensor_mul(out=ht_acc[:], in0=o_s[:], in1=tnh_c[:])

                nc.sync.dma_start(out=ht[:, d0:d0 + sc_cols], in_=ht_acc[:])
                nc.sync.dma_start(out=ct[:, d0:d0 + sc_cols], in_=ct_acc[:])

    nc.compile()
    return nc


def _prep_core_inputs(core, inputs, st, geom, cells):
    x_l = inputs["x_l"]
    h0, c0 = inputs["h0"], inputs["c0"]
    n_sc, scc = st["n_sc"], st["sc_chunks"]
    budgets = st["budgets"]
    bank_rows, n_lit = st["bank_rows"], st["n_lit"]
    ncols = st["n_chunks"] * P
    cpc = CPC
    c_lo = core * cpc

    h0t = np.zeros((P, ncols), dtype=np.float16)
    c0t = np.zeros((P, ncols), dtype=np.float16)
    h0t[:, :cpc] = h0[c_lo:c_lo + cpc].T.astype(np.float16)
    c0t[:, :cpc] = c0[c_lo:c_lo + cpc].T.astype(np.float16)

    eidx = np.zeros((P, geom["total_icols"]), dtype=np.int16)
    eloc_flat = np.full(geom["total_tiles"] * P, -1.0, dtype=np.float16)

    for s, (start, scc_s, _groups) in enumerate(_sc_list(st)):
        g = geom["sc_geom"][s]
        # global tile index of bank b's first tile in this sc
        bank_base = {}
        for ti, (b, t, c) in enumerate(g["tiles"]):
            if b not in bank_base:
                bank_base[b] = g["tbase"] + ti
        for b, icol0, cnidx, t0, ntile in g["calls"]:
            rows = min(bank_rows, n_lit - b * bank_rows)
            flat = (np.arange(cnidx, dtype=np.int64) * 97 % rows).astype(np.int16)
            off = 0       # slot offset within this call
            t_seen = 0    # tiles of bank b seen so far (cell/chunk order)
            last_fill = 0  # end of the last real edge in this call
            for c in range(scc_s):
                j = start + c
                bud = int(budgets[j, b])
                if bud == 0:
                    continue
                nt_cell = bud // P
                if t_seen + nt_cell <= t0:
                    t_seen += nt_cell
                    continue
                if t_seen >= t0 + ntile:
                    break
                lits, locs = cells.get((j, b), (None, None))
                if lits is not None:
                    n = len(lits)
                    flat[off:off + n] = lits
                    last_fill = off + n
                    gslot0 = (bank_base[b] + t_seen) * P
                    eloc_flat[gslot0:gslot0 + n] = locs
                off += bud
                t_seen += nt_cell
            # the gather ucode trims trailing negative indices before
            # descriptor generation: mark the call's tail padding as -1
            # (the skipped slots stay zero/stale-finite; their one-hot
            # columns are all-zero so they contribute nothing).
            if last_fill < cnidx and False:  # BISECT: trim disabled
                flat[last_fill:] = -1
            blk = flat.reshape(cnidx // 16, 16).T
            for r in range(8):
                eidx[16 * r:16 * (r + 1), icol0:icol0 + cnidx // 16] = blk
    eloc1 = eloc_flat.reshape(geom["total_tiles"], P).T  # [P, total_tiles]
    # each value duplicated: eloc[p, 2t] == eloc[p, 2t+1] (pair-trick build)
    eloc = np.repeat(eloc1, 2, axis=1).copy()

    return {"xlb": np.ascontiguousarray(x_l.astype(np.float16)),
            "h0t": h0t, "c0t": c0t, "eidx": eidx, "eloc": eloc}


def _shared_inputs(inputs, geom):
    W_ih, W_hh = inputs["W_ih"], inputs["W_hh"]
    b2 = (inputs["b_ih"] + inputs["b_hh"]).astype(np.float32)
    wih = np.ascontiguousarray(W_ih.T.astype(np.float16))
    whh = np.ascontiguousarray(W_hh.T.astype(np.float16))
    btab = np.ascontiguousarray(b2.reshape(4, P).T)
    iota = np.broadcast_to(np.arange(P, dtype=np.float16), (P, P))
    return {"wih": wih, "whh": whh, "btab": btab,
            "iotaf": np.ascontiguousarray(iota)}


def kernel(x_l, h0, c0, W_ih, W_hh, b_ih, b_hh, edge_lit, edge_clause):
    from concourse.bass_utils import run_bass_kernel_spmd

    inputs = dict(x_l=x_l, h0=h0, c0=c0, W_ih=W_ih, W_hh=W_hh, b_ih=b_ih,
                  b_hh=b_hh, edge_lit=edge_lit, edge_clause=edge_clause)

    st, per_core = _compute_structure(np.asarray(edge_lit),
                                      np.asarray(edge_clause))
    geom = _geometry(st)
    key = ("v7", st["n_chunks"], st["n_banks"], st["budgets"].tobytes())
    if key not in _cache:
        _cache[key] = _build_program(st, geom)
    nc = _cache[key]

    shared = _shared_inputs(inputs, geom)
    in_maps = []
    for k in range(N_CORES):
        m = _prep_core_inputs(k, inputs, st, geom, per_core[k])
        m.update(shared)
        in_maps.append(m)

    res = run_bass_kernel_spmd(nc, in_maps, core_ids=list(range(N_CORES)),
                               trace=TRACE)
    global LAST_RESULT
    LAST_RESULT = res

    h_new = np.empty((N_CLAUSE, D), dtype=np.float32)
    c_new = np.empty((N_CLAUSE, D), dtype=np.float32)
    for k in range(N_CORES):
        out = res.results[k]
        h_new[k * CPC:(k + 1) * CPC] = out["ht"][:, :CPC].T.astype(np.float32)
        c_new[k * CPC:(k + 1) * CPC] = out["ct"][:, :CPC].T.astype(np.float32)
    return (h_new, c_new)

